# revision 1
# baseline (speedup 1.0000x reference)
"""CapsuleLayer (dynamic routing) Trainium2 kernel.

Full inputs:  x (32, 2048, 32) f32, W (2048, 64, 32, 32) f32  [W indexed n,j,d,k]
Output:       v (32, 64, 32) f32

Math (reference):
    u_hat[b,j,n,k] = sum_d W[n,j,d,k] * x[b,n,d]
    b = 0; 3 routing iters:
        c = softmax_j(b); s[b,j,k] = sum_n c[b,j,n]*u_hat[b,j,n,k]; v = squash(s)
        b += sum_k u_hat[b,j,n,k]*v[b,j,k]   (first 2 iters)

Sharding: input-capsule axis n split over 8 cores (256 each). Per core:
  - W slice packed on host to fp16 tiles [g, (n4 d), (j k)]  (g = group of 4 n)
  - x packed on host to fp16: dense [(n4 d), (g b)] for the s1 matmuls and
    block-diagonal [(n4 d), (g, (n4' b))] for the u_hat matmuls
  - pass 1: u_hat = blockdiag(x).T @ W on PE (full 128-contraction, N=512
    chunks) into PSUM; copied to fp16 SBUF (resident groups, spread evenly)
    or DRAM (spilled); s1 = dense(x).T @ W accumulated in PSUM across groups
  - routing passes: per group, w = u*v_rep (GPSIMD, every 5th on DVE);
    t = reduce_k(w) (DVE); b += t; softmax normalized without reciprocal:
    e=exp(b-10) with ACT accum_out -> se; c = exp(b - 10 - log se) emitted
    directly as the k-broadcast c_rep in one ACT op; cu = c_rep*u (DVE);
    s += odiag.T @ cu on PE (block-diag ones collapses the 4 n per tile)
  - iterations 1,2: 256KB AllReduce of s over 8 cores, squash via
    s2*exp(-ln(1+s2)-0.5*ln(s2+eps)) (no reciprocal/sqrt table swaps),
    v replicated to the (n4 b) partition layout in fp16
  - iteration 3: each core ships its partial s3; host sums and squashes
  - single ACT table set (natural_log_exp_and_others) pinned via a Bacc
    subclass so the per-group Exp/Ln/Copy chain never reloads ACT tables
"""

import os
from contextlib import ExitStack

import numpy as np

B, NTOT, DD, J, K = 32, 2048, 32, 64, 32
JK = J * K
CORES = 8
NL = NTOT // CORES          # input capsules per core
ITERS = 3

_CACHED = {}


def _build_nc(NL_, G_RES, n_cores, repeat=1):
    import concourse.bass as bass
    import concourse.mybir as mybir
    import concourse.tile as tile
    from concourse import bacc
    from concourse.masks import make_identity

    G = NL_ // 4            # groups of 4 input capsules
    G_RES = min(G_RES, G)
    NSPILL = G - G_RES
    f16 = mybir.dt.float16
    f32 = mybir.dt.float32
    AX = mybir.AxisListType
    OP = mybir.AluOpType
    AF = mybir.ActivationFunctionType

    import bass_rust as _bass_rust
    from concourse.hw_specs import get_activation_tables

    class _CapsBacc(bacc.Bacc):
        # Keep only a covering act-table set so the per-group Exp/Ln/Copy
        # chain never reloads ACT tables (the stock pass thrashes sets).
        _ACT_KEEP = {"natural_log_exp_and_others", "sqrt_and_others"}

        def insert_act_table_loads(self):
            has_act = any(
                isinstance(i, mybir.InstActivation)
                for bb in self.main_func.blocks for i in bb.instructions
            )
            if not has_act:
                return
            tables = [
                (n, (f if n in self._ACT_KEEP else set()))
                for n, f in get_activation_tables(self.m.arch).items()
            ]
            _bass_rust.insert_act_table_loads(self, tables)

    nc = _CapsBacc()
    wd = nc.declare_dram_parameter("w", [G, 128, JK], f16, isOutput=False)
    xtd = nc.declare_dram_parameter("xt", [128, G * B], f16, isOutput=False)
    xbd_d = nc.declare_dram_parameter("xb", [128, G * 128], f16, isOutput=False)
    od_d = nc.declare_dram_parameter("od", [128, B], f16, isOutput=False)
    vd = nc.declare_dram_parameter("v", [B, JK], f32, isOutput=True)

    core_ids = list(range(n_cores))

    with tile.TileContext(nc) as tc, ExitStack() as ctx:
        const = ctx.enter_context(tc.tile_pool(name="const", bufs=1))
        dram = ctx.enter_context(tc.tile_pool(name="dram", bufs=1, space="DRAM"))
        ures = ctx.enter_context(tc.tile_pool(name="ures", bufs=1))
        sm = ctx.enter_context(tc.tile_pool(name="small", bufs=1))
        smg = ctx.enter_context(tc.tile_pool(name="smallg", bufs=8))
        sv = ctx.enter_context(tc.tile_pool(name="sv", bufs=2))
        vrp = ctx.enter_context(tc.tile_pool(name="vrp", bufs=2))

        # ---- constants ----
        xts = const.tile([128, G * B], f16)
        nc.sync.dma_start(out=xts, in_=xtd[:])
        xbd = const.tile([128, G * 128], f16)   # block-diag x per group
        nc.sync.dma_start(out=xbd, in_=xbd_d[:])
        odiag = const.tile([128, B], f16)   # odiag[p, b] = 1 if p % 32 == b
        nc.sync.dma_start(out=odiag, in_=od_d[:])
        bm10 = const.tile([128, 1], f32)
        nc.vector.memset(bm10, -10.0)
        beps = const.tile([128, 1], f32)
        nc.vector.memset(beps, 1e-8)

        b_sb = const.tile([128, G * J], f32)        # routing logits per (n4 b)

        if NSPILL:
            u_spill = dram.tile([NSPILL, 128, JK], f16)
        cc_in = dram.tile([B, JK], f32)
        cc_out = dram.tile([B, JK], f32)

        u_tiles = {}
        res_set = {g for g in range(G) if (g * G_RES) % G < G_RES}
        spill_idx = {}
        for g in range(G):
            if g not in res_set:
                spill_idx[g] = len(spill_idx)

        def u_tile(g):
            if g in res_set:
                if g not in u_tiles:
                    u_tiles[g] = ures.tile(
                        [128, JK], f16, tag=f"u{g}", name=f"u{g}"
                    )
                return u_tiles[g], True
            return None, False

        # ---------- squash + AllReduce of s; returns v_rep fp16 [128, JK] ----------
        def finish_iteration(s_psum, scale_mul, last):
            s_sb = sm.tile([B, JK], f32, tag="s_work")
            nc.scalar.mul(s_sb, s_psum, scale_mul)
            if last:
                # host gathers per-core partial s and finishes squash there
                nc.sync.dma_start(out=vd[:], in_=s_sb)
                return None
            nc.sync.dma_start(out=cc_in[:], in_=s_sb)
            nc.gpsimd.collective_compute(
                "AllReduce",
                OP.add,
                ins=[cc_in[:].opt()],
                outs=[cc_out[:].opt()],
                replica_groups=[core_ids],
            )
            s_tot = sm.tile([B, JK], f32, tag="s_work", name="s_tot")
            nc.sync.dma_start(out=s_tot, in_=cc_out[:])

            # squash scale: sc = s2/(1+s2)/sqrt(s2+eps)
            #             = s2 * exp(-ln(1+s2) - 0.5*ln(s2+eps))
            sq = sm.tile([B, JK], f32, tag="tmp1")
            nc.vector.tensor_mul(sq, s_tot, s_tot)
            s2 = sm.tile([B, J], f32, tag="s2")
            nc.vector.tensor_reduce(
                s2, sq.rearrange("b (j k) -> b j k", j=J), axis=AX.X, op=OP.add
            )
            a_ln = sm.tile([B, J], f32, tag="a_ln")
            nc.scalar.activation(a_ln, s2, AF.Ln, bias=1.0, scale=1.0)
            b_ln = sm.tile([B, J], f32, tag="b_ln")
            nc.scalar.activation(b_ln, s2, AF.Ln, bias=beps[:B], scale=1.0)
            comb = sm.tile([B, J], f32, tag="comb")
            nc.vector.scalar_tensor_tensor(
                comb, b_ln, -0.5, a_ln, op0=OP.mult, op1=OP.subtract
            )
            e_sc = sm.tile([B, J], f32, tag="e_sc")
            nc.scalar.activation(e_sc, comb, AF.Exp)
            sc = sm.tile([B, J], f32, tag="sc")
            nc.vector.tensor_mul(sc, s2, e_sc)
            sc_rep = sm.tile([B, J, K], f32, tag="tmp2")
            sc_b = bass.AP(
                tensor=sc.tensor, offset=sc.offset,
                ap=[sc.ap[0], sc.ap[1], [0, K]],
            )
            nc.scalar.copy(sc_rep, sc_b)
            scr = sc_rep.rearrange("b j k -> b (j k)")
            v_rep = vrp.tile([128, JK], f16, tag="v_rep")
            for r in range(4):
                rs = slice(32 * r, 32 * r + 32)
                if r % 2 == 0:
                    nc.vector.tensor_mul(v_rep[rs, :], s_tot, scr)
                else:
                    nc.gpsimd.tensor_mul(v_rep[rs, :], s_tot, scr)
            return v_rep

        # ================= pass 1: u_hat + s1 =================
        for rep in range(repeat):
          with tc.tile_pool(name=f"wp{rep}", bufs=4) as wp, \
             tc.tile_pool(name=f"pu{rep}", bufs=2, space="PSUM") as pu, \
             tc.tile_pool(name=f"ps1{rep}", bufs=1, space="PSUM") as ps1, \
             tc.tile_pool(name=f"ustg1{rep}", bufs=3) as ustg1:
              s1_psum = ps1.tile([B, JK], f32)
              for g in range(G):
                  wt = wp.tile([128, JK], f16, tag="wt")
                  nc.sync.dma_start(out=wt, in_=wd[g])
                  ut, resident = u_tile(g)
                  if not resident:
                      ut = ustg1.tile([128, JK], f16, tag="ustg")
                  xsl = xts[:, g * B:(g + 1) * B]
                  xbsl = xbd[:, g * 128:(g + 1) * 128]
                  for h in range(2):
                      up = pu.tile([128, 1024], f32, tag="up")
                      for cch in range(2):
                          lo = h * 1024 + cch * 512
                          sl = slice(lo, lo + 512)
                          psl = slice(cch * 512, cch * 512 + 512)
                          nc.tensor.matmul(
                              up[:, psl],
                              lhsT=xbsl,
                              rhs=wt[:, sl],
                              start=True, stop=True,
                              skip_group_check=True,
                          )
                          nc.tensor.matmul(
                              s1_psum[:, sl],
                              lhsT=xsl,
                              rhs=wt[:, sl],
                              start=(g == 0), stop=(g == G - 1),
                              skip_group_check=True,
                          )
                      osl = slice(h * 1024, h * 1024 + 1024)
                      if h == 0:
                          nc.vector.tensor_copy(ut[:, osl], up)
                      else:
                          nc.scalar.copy(ut[:, osl], up)
                  if not resident:
                      nc.scalar.dma_start(out=u_spill[spill_idx[g]], in_=ut)
              v_rep = finish_iteration(s1_psum, 1.0 / J, last=False)

          # ================= passes 2..ITERS =================
          with tc.tile_pool(name=f"ps23{rep}", bufs=1, space="PSUM") as ps23, \
             tc.tile_pool(name=f"ustg2{rep}", bufs=5) as ustg2, \
             tc.tile_pool(name=f"wtp{rep}", bufs=3) as wtp, \
             tc.tile_pool(name=f"crp{rep}", bufs=3) as crp, \
             tc.tile_pool(name=f"cup{rep}", bufs=3) as cup:
              for it in range(1, ITERS):
                  s_psum = ps23.tile([B, JK], f32, tag="s23")
                  for g in range(G):
                      ut, resident = u_tile(g)
                      if not resident:
                          ut = ustg2.tile([128, JK], f16, tag="ustg2")
                          nc.sync.dma_start(out=ut, in_=u_spill[spill_idx[g]])
                      w_t = wtp.tile([128, JK], f16, tag="w_t")
                      if g % 5 == 4:
                          nc.vector.tensor_mul(w_t, ut, v_rep)
                      else:
                          nc.gpsimd.tensor_mul(w_t, ut, v_rep)
                      bsl = b_sb[:, g * J:(g + 1) * J]
                      if it == 1:
                          nc.vector.tensor_reduce(
                              bsl, w_t.rearrange("p (j k) -> p j k", j=J),
                              axis=AX.X, op=OP.add,
                          )
                      else:
                          t_t = smg.tile([128, J], f32, tag="t_t")
                          nc.vector.tensor_reduce(
                              t_t, w_t.rearrange("p (j k) -> p j k", j=J),
                              axis=AX.X, op=OP.add,
                          )
                          nc.vector.tensor_add(bsl, bsl, t_t)
                      e_t = smg.tile([128, J], f32, tag="e_t")
                      se = smg.tile([128, 1], f32, tag="se")
                      # e = exp(b - 10); se = sum_j e   (one ACT op)
                      nc.scalar.activation(
                          e_t, bsl, AF.Exp, bias=bm10, scale=1.0,
                          accum_out=se,
                      )
                      # nls = -log(se) - 10  =>  c = exp(b + nls - ...) wait:
                      # c = exp(b - 10 - log se) = e / se  (exact softmax)
                      ls = smg.tile([128, 1], f32, tag="ls")
                      nc.scalar.activation(ls, se, AF.Ln)
                      nls = smg.tile([128, 1], f32, tag="nls")
                      nc.scalar.activation(
                          nls, ls, AF.Copy, bias=-10.0, scale=-1.0
                      )
                      c_rep = crp.tile([128, J, K], f16, tag="c_rep")
                      b_b = bass.AP(
                          tensor=bsl.tensor, offset=bsl.offset,
                          ap=[bsl.ap[0], bsl.ap[1], [0, K]],
                      )
                      nc.scalar.activation(
                          c_rep, b_b, AF.Exp, bias=nls, scale=1.0
                      )
                      cu = cup.tile([128, JK], f16, tag="cu")
                      nc.vector.tensor_mul(
                          cu, c_rep.rearrange("p j k -> p (j k)"), ut
                      )
                      for cch in range(4):
                          sl = slice(cch * 512, cch * 512 + 512)
                          nc.tensor.matmul(
                              s_psum[:, sl],
                              lhsT=odiag,
                              rhs=cu[:, sl],
                              start=(g == 0), stop=(g == G - 1),
                              skip_group_check=True,
                          )
                  v_rep = finish_iteration(
                      s_psum, 1.0, last=(it == ITERS - 1)
                  )

    nc.finalize()
    return nc


def _pack_inputs(x, W, n_cores, ntot=NTOT):
    """Shard over n, cast fp16, pre-transpose to the on-chip layouts."""
    nl = ntot // n_cores
    g = nl // 4
    in_maps = []
    for c in range(n_cores):
        wl = W[c * nl:(c + 1) * nl]                       # (nl, J, D, K)
        wp = np.ascontiguousarray(
            wl.reshape(g, 4, J, DD, K).transpose(0, 1, 3, 2, 4)
            .reshape(g, 128, JK).astype(np.float16)
        )
        xl = x[:, c * nl:(c + 1) * nl, :]                 # (B, nl, D)
        xg = xl.transpose(1, 2, 0).reshape(g, 4, DD, B).astype(np.float16)
        xt = np.ascontiguousarray(
            xg.reshape(g, 128, B).transpose(1, 0, 2)      # (128, g, b)
            .reshape(128, g * B)
        )
        xb = np.zeros((g, 128, 128), np.float16)
        for ns in range(4):
            xb[:, ns * 32:(ns + 1) * 32, ns * 32:(ns + 1) * 32] = xg[:, ns]
        xb = np.ascontiguousarray(
            xb.transpose(1, 0, 2).reshape(128, g * 128)
        )
        od = np.tile(np.eye(32, dtype=np.float16), (4, 1))
        in_maps.append({"w": wp, "xt": xt, "xb": xb, "od": od})
    return in_maps


def kernel(x, W):
    from concourse.bass_utils import run_bass_kernel_spmd

    x = np.asarray(x, dtype=np.float32)
    W = np.asarray(W, dtype=np.float32)
    g_res = int(os.environ.get("CAPS_G_RES", "16"))
    key = (NL, g_res, CORES)
    if key not in _CACHED:
        _CACHED[key] = _build_nc(NL, g_res, CORES)
    nc = _CACHED[key]
    in_maps = _pack_inputs(x, W, CORES)
    res = run_bass_kernel_spmd(nc, in_maps, list(range(CORES)))
    s = np.zeros((B, JK), np.float32)
    for c in range(CORES):
        s += np.asarray(res.results[c]["v"], dtype=np.float32)
    s = s.reshape(B, J, K)
    s2 = np.sum(s * s, axis=-1, keepdims=True)
    v = s2 / (1.0 + s2) / np.sqrt(s2 + 1e-8) * s
    return v.astype(np.float32)

